# revision 3
# baseline (speedup 1.0000x reference)
"""Trainium2 Bass kernel for nn_Action_Prediction (segment_reduce).

Computation (reference):
  logits = MLP(X)  with layers 128->256->256->256->1 (ReLU between)
  per-segment (4096 segments of exactly 128 contiguous nodes):
    softmax over the segment, Gumbel-max sample (fixed key 42),
    outputs (p[B], actions[B], shifted_actions[B]).

Strategy: data-parallel over nodes across 8 NeuronCores (65536 nodes each).
X is transposed + cast to fp16 on the host so each core DMAs [feat=128,
node] tiles; the whole MLP runs with transposed activations [H, node].

fp16 everywhere in the MLP: fp16 has the same 11-bit mantissa as the
TF32-like f32r mode the previous kernel used (host simulation shows
max logits error 3.3e-4 against f64, 0/4096 argmax flips with the
min per-segment top-2 score gap at 2.9e-4), but unlike f32r it enables
Fast Weight Load so the per-matmul LDWEIGHTS (~107 ns for f32) hides
behind the matmul, and it halves X's DMA traffic.  Matmul free dim is
512 (fp16 moving max is 1024) so the NX issue overhead amortizes:
steady-state target ~215.8 ns per [128,128,512] matmul, 12 matmuls per
512-node tile = 12 cycles/node, ~331 us/core stream.

PSUM (8 banks): ph0/ph1/ph2 [128,1024] f32 = 2 banks each (bufs=1),
plg [1,1024] = 2 banks.  A short run of warmup matmuls on a memset
tile runs during the initial DMA wait to flip the PE HAM clock gate
(1.2 -> 2.4 GHz) before the real stream starts.

Logits strip is laid out [segment, node]: per 2-tile group the Lf
matmuls accumulate 1024 logits in plg (partition 0), one DVE copy
evacuates to SBUF and one DMA scatters them to 8 strip rows (one per
segment).  Per-segment softmax/argmax then runs on 4 chunks of
[128 segments, 128 nodes] with per-partition scalars:
  sc = strip+g; m = rowmax(sc); e,S = exp(strip) with accumulate;
  scr = (sc>=m)*iota -> am = rowmax (max-index tie-break, same as
  reference); ewin = sum((iota==am)*e); p = ewin/S.
Chunks 0-2 run during the matmul stream; only chunk 3 is tail work.
actions = am (local index, segments are exactly 128 nodes); host
derives shifted_actions = 128*seg + action.
"""

import sys

if "/opt/trn_rl_repo" not in sys.path:
    sys.path.insert(0, "/opt/trn_rl_repo")

import numpy as np

import concourse.bacc as bacc
import concourse.mybir as mybir
from concourse import tile
from concourse.bass_utils import run_bass_kernel_spmd

F32 = mybir.dt.float32
F16 = mybir.dt.float16
I32 = mybir.dt.int32
AF = mybir.ActivationFunctionType
OP = mybir.AluOpType
AX = mybir.AxisListType

N_CORES = 8
N = 524288
D = 128
H = 256
B_SEG = 4096
SEG = 128            # nodes per segment
N_LOC = N // N_CORES         # 65536 nodes per core
SEG_LOC = N_LOC // SEG       # 512 segments per core
CH = 128                     # segments per chunk (= partition dim)
NCHUNK = SEG_LOC // CH       # 4
NT_FULL = 128                # kept for test.py compat (tiles at tw=512)
TW = 512


def build(nt=NT_FULL, tw=TW, warmups=18):
    nc = bacc.Bacc("TRN2", target_bir_lowering=False, debug=False)
    ntile = N_LOC // tw          # tiles per core
    gtiles = 1024 // tw          # tiles per logits group (1024 nodes)
    ngrp = ntile // gtiles       # 64 groups
    grp_per_chunk = ngrp // NCHUNK   # 16
    pbufs = 1 if tw > 256 else 2

    xt_d = nc.dram_tensor("xt", [128, N_LOC], F16, kind="ExternalInput")
    w0_d = nc.dram_tensor("w0", [128, 256], F16, kind="ExternalInput")
    wc_d = nc.dram_tensor("wc", [128, 1026], F16, kind="ExternalInput")
    g_d = nc.dram_tensor("g", [128, 512], F32, kind="ExternalInput")
    out_d = nc.dram_tensor("out", [128, 8], F32, kind="ExternalOutput")

    with tile.TileContext(nc) as tc:
        with tc.tile_pool(name="const", bufs=1) as cpool, \
             tc.tile_pool(name="xp", bufs=5) as xpool, \
             tc.tile_pool(name="hp", bufs=2) as hpool, \
             tc.tile_pool(name="pp", bufs=1, space="PSUM") as ppool, \
             tc.tile_pool(name="sp", bufs=2) as spool:

            w0t = cpool.tile([128, 256], F16)
            wct = cpool.tile([128, 1026], F16)
            gt = cpool.tile([128, 512], F32)
            strip = cpool.tile([128, 512], F32)
            out_t = cpool.tile([128, 8], F32)
            warm16 = cpool.tile([128, 128], F16)
            nc.gpsimd.memset(warm16[:], 0)

            # Head DMAs: w0 on the Sync queue, x0 in parallel on the
            # Scalar (Activation) HW-DGE queue so the first matmul's two
            # dependencies transfer concurrently.
            nc.sync.dma_start(w0t[:], w0_d[:])
            xts = {}
            xts[0] = xpool.tile([128, tw], F16, tag="xt", name="xt0")
            nc.scalar.dma_start(xts[0][:], xt_d[:, 0:tw])
            xts[1] = xpool.tile([128, tw], F16, tag="xt", name="xt1")
            nc.sync.dma_start(xts[1][:], xt_d[:, tw:2 * tw])
            nc.sync.dma_start(wct[:], wc_d[:])
            w1 = wct[:, 0:512]
            w2 = wct[:, 512:1024]
            wf = wct[:, 1024:1026]

            iota_i = cpool.tile([128, 128], I32)
            nc.gpsimd.iota(iota_i[:], pattern=[[1, 128]], base=0,
                           channel_multiplier=0)
            iota128 = cpool.tile([128, 128], F32)
            nc.vector.tensor_copy(iota128[:], iota_i[:])

            # PE warmup: dependency-free matmuls fill the initial DMA
            # wait so the HAM clock gate is already 8/8 when the real
            # stream starts.  Reuses the ph2 PSUM allocation.
            wps = ppool.tile([128, 2 * tw], F32, tag="ph2", bufs=pbufs,
                             name="warm")
            for i in range(warmups):
                nc.tensor.matmul(wps[:, 0:128], warm16[:], warm16[:],
                                 start=True, stop=True)

            def emit_chunk(c):
                cs = CH * c
                stc = strip[:, cs:cs + CH]
                gc = gt[:, cs:cs + CH]
                sc = spool.tile([128, CH], F32, tag="sc", name=f"sc{c}")
                nc.vector.tensor_add(sc[:], stc, gc)
                e_c = spool.tile([128, CH], F32, tag="ec", name=f"ec{c}")
                s_c = spool.tile([128, 1], F32, tag="Sc", name=f"Sc{c}")
                nc.scalar.activation(e_c[:], stc, AF.Exp, accum_out=s_c[:])
                m_c = spool.tile([128, 1], F32, tag="mc", name=f"mc{c}")
                nc.vector.tensor_reduce(m_c[:], sc[:], axis=AX.X, op=OP.max)
                scr = spool.tile([128, CH], F32, tag="scr", name=f"scr{c}")
                nc.vector.scalar_tensor_tensor(scr[:], sc[:], m_c[:],
                                               iota128[:], op0=OP.is_ge,
                                               op1=OP.mult)
                nc.vector.tensor_reduce(out_t[:, 4 + c:5 + c], scr[:],
                                        axis=AX.X, op=OP.max)
                ew = spool.tile([128, CH], F32, tag="ew", name=f"ew{c}")
                ewin = spool.tile([128, 1], F32, tag="ewin", name=f"ewin{c}")
                nc.vector.scalar_tensor_tensor(ew[:], iota128[:],
                                               out_t[:, 4 + c:5 + c],
                                               e_c[:], op0=OP.is_equal,
                                               op1=OP.mult,
                                               accum_out=ewin[:])
                rcp = spool.tile([128, 1], F32, tag="rcp", name=f"rcp{c}")
                nc.vector.reciprocal(rcp[:], s_c[:])
                nc.vector.tensor_mul(out_t[:, c:c + 1], ewin[:], rcp[:])

            h0s = {}
            plg = None
            for tt in range(ntile + 1):
                if tt < ntile:
                    t = tt
                    if t in xts:
                        xt = xts.pop(t)
                    else:
                        xt = xpool.tile([128, tw], F16, tag="xt",
                                        name=f"xt{t}")
                        nc.sync.dma_start(xt[:], xt_d[:, t * tw:(t + 1) * tw])
                    if t == 4:
                        nc.sync.dma_start(gt[:], g_d[:])
                    ph0 = ppool.tile([128, 2 * tw], F32, tag="ph0",
                                     bufs=pbufs, name=f"ph0_{t}")
                    nc.tensor.matmul(ph0[:, 0:tw], w0t[:, 0:128], xt[:],
                                     start=True, stop=True)
                    nc.tensor.matmul(ph0[:, tw:2 * tw], w0t[:, 128:256],
                                     xt[:], start=True, stop=True)
                    h0 = hpool.tile([128, 2 * tw], F16, tag="h0", bufs=2,
                                    name=f"h0_{t}")
                    nc.scalar.activation(h0[:], ph0[:], AF.Relu)
                    h0s[t] = h0
                if tt < 1:
                    continue
                t = tt - 1
                h0 = h0s.pop(t)
                h0a, h0b = h0[:, 0:tw], h0[:, tw:2 * tw]

                ph1 = ppool.tile([128, 2 * tw], F32, tag="ph1", bufs=pbufs,
                                 name=f"ph1_{t}")
                nc.tensor.matmul(ph1[:, 0:tw], w1[:, 0:128], h0a,
                                 start=True, stop=False)
                nc.tensor.matmul(ph1[:, 0:tw], w1[:, 256:384], h0b,
                                 start=False, stop=True)
                nc.tensor.matmul(ph1[:, tw:2 * tw], w1[:, 128:256], h0a,
                                 start=True, stop=False)
                nc.tensor.matmul(ph1[:, tw:2 * tw], w1[:, 384:512], h0b,
                                 start=False, stop=True)
                h1 = hpool.tile([128, 2 * tw], F16, tag="h1", bufs=2,
                                name=f"h1_{t}")
                nc.vector.tensor_scalar(h1[:], ph1[:], 0.0, None, OP.max)
                h1a, h1b = h1[:, 0:tw], h1[:, tw:2 * tw]

                ph2 = ppool.tile([128, 2 * tw], F32, tag="ph2", bufs=pbufs,
                                 name=f"ph2_{t}")
                nc.tensor.matmul(ph2[:, 0:tw], w2[:, 0:128], h1a,
                                 start=True, stop=False)
                nc.tensor.matmul(ph2[:, 0:tw], w2[:, 256:384], h1b,
                                 start=False, stop=True)
                nc.tensor.matmul(ph2[:, tw:2 * tw], w2[:, 128:256], h1a,
                                 start=True, stop=False)
                nc.tensor.matmul(ph2[:, tw:2 * tw], w2[:, 384:512], h1b,
                                 start=False, stop=True)
                h2 = hpool.tile([128, 2 * tw], F16, tag="h2", bufs=2,
                                name=f"h2_{t}")
                nc.scalar.activation(h2[:], ph2[:], AF.Relu)
                h2a, h2b = h2[:, 0:tw], h2[:, tw:2 * tw]

                if t % gtiles == 0:
                    plg = ppool.tile([1, 1024], F32, tag="plg", bufs=1,
                                     name=f"plg{t}")
                c0 = (t % gtiles) * tw
                nc.tensor.matmul(plg[0:1, c0:c0 + tw], wf[:, 0:1], h2a,
                                 start=True, stop=False)
                nc.tensor.matmul(plg[0:1, c0:c0 + tw], wf[:, 1:2], h2b,
                                 start=False, stop=True)
                if t % gtiles == gtiles - 1:
                    gidx = t // gtiles
                    stage = spool.tile([1, 1024], F32, tag="stage", bufs=2,
                                       name=f"stg{gidx}")
                    nc.vector.tensor_copy(stage[:], plg[:])
                    c = gidx // grp_per_chunk
                    r = 8 * (gidx % grp_per_chunk)
                    nc.sync.dma_start(strip[r:r + 8, CH * c:CH * (c + 1)],
                                      stage[0:1, :])
                    if gidx % grp_per_chunk == grp_per_chunk - 1:
                        emit_chunk(c)

            nc.sync.dma_start(out_d[:], out_t[:])

    nc.compile()
    return nc


_NC_CACHE = {}


def _get_nc(nt=NT_FULL):
    if nt not in _NC_CACHE:
        _NC_CACHE[nt] = build(nt)
    return _NC_CACHE[nt]


def _gumbel_host():
    import jax

    with jax.default_device(jax.devices("cpu")[0]):
        skey = jax.random.key(42)
        u = jax.random.uniform(skey, (N,), np.float32, 1e-20, 1.0)
        g = -np.log(-np.log(np.asarray(u)))
    return g.astype(np.float32)


def prep_in_maps(X, W0, b0, W1, b1, W2, b2, Wf, bf, g=None):
    # the graph folds the (always-zero) biases away; fail loudly otherwise
    for b in (b0, b1, b2):
        assert not np.any(np.asarray(b)), "nonzero MLP biases unsupported"
    if g is None:
        g = _gumbel_host()
    W1 = np.asarray(W1, np.float32)
    W2 = np.asarray(W2, np.float32)
    Wf = np.asarray(Wf, np.float32)
    w0 = np.asarray(W0, np.float32).astype(np.float16)          # [128,256]
    w1 = np.concatenate([W1[:128], W1[128:]], 1).astype(np.float16)
    w2 = np.concatenate([W2[:128], W2[128:]], 1).astype(np.float16)
    wf = np.stack([Wf[:128, 0], Wf[128:, 0]], 1).astype(np.float16)
    wc = np.ascontiguousarray(np.concatenate([w1, w2, wf], 1))  # [128,1026]
    X16 = np.asarray(X, np.float32).astype(np.float16)
    in_maps = []
    for c in range(N_CORES):
        xc = X16[c * N_LOC:(c + 1) * N_LOC]
        xtc = np.ascontiguousarray(xc.T)                        # [128,65536]
        gc = np.ascontiguousarray(
            g[c * N_LOC:(c + 1) * N_LOC]
            .reshape(NCHUNK, CH, SEG).transpose(1, 0, 2).reshape(128, 512))
        in_maps.append({
            "xt": xtc, "w0": np.ascontiguousarray(w0), "wc": wc, "g": gc,
        })
    return in_maps


def assemble(results):
    p = np.empty(B_SEG, np.float32)
    actions = np.empty(B_SEG, np.int32)
    for c in range(N_CORES):
        o = results[c]["out"]  # [128, 8]
        for cc in range(NCHUNK):
            lo = c * SEG_LOC + cc * CH
            p[lo:lo + CH] = o[:, cc]
            actions[lo:lo + CH] = np.rint(o[:, 4 + cc]).astype(np.int32)
    shifted = (np.arange(B_SEG, dtype=np.int32) * SEG + actions).astype(
        np.int32)
    return p, actions, shifted


def kernel(X, W0, b0, W1, b1, W2, b2, Wf, bf, batch, **kwargs):
    nc = _get_nc()
    in_maps = prep_in_maps(X, W0, b0, W1, b1, W2, b2, Wf, bf)
    res = run_bass_kernel_spmd(nc, in_maps, core_ids=list(range(N_CORES)))
    return assemble(res.results)


# revision 6
# speedup vs baseline: 1.1503x; 1.1503x over previous
"""Trainium2 Bass kernel for nn_Action_Prediction (segment_reduce).

Computation (reference):
  logits = MLP(X)  with layers 128->256->256->256->1 (ReLU between)
  per-segment (4096 segments of exactly 128 contiguous nodes):
    softmax over the segment, Gumbel-max sample (fixed key 42),
    outputs (p[B], actions[B], shifted_actions[B]).

Strategy: data-parallel over nodes across 8 NeuronCores (65536 nodes each).
X is transposed + cast to fp16 on the host so each core DMAs [feat=128,
node] tiles; the whole MLP runs with transposed activations [H, node].

fp16 everywhere in the MLP: fp16 has the same 11-bit mantissa as the
TF32-like f32r mode the previous kernel used (host simulation shows
max logits error 3.3e-4 against f64, 0/4096 argmax flips with the
min per-segment top-2 score gap at 2.9e-4), but unlike f32r it enables
Fast Weight Load so the per-matmul LDWEIGHTS (~107 ns for f32) hides
behind the matmul, and it halves X's DMA traffic.  Matmul free dim is
512 (fp16 moving max is 1024) so the NX issue overhead amortizes:
steady-state target ~215.8 ns per [128,128,512] matmul, 12 matmuls per
512-node tile = 12 cycles/node, ~331 us/core stream.

PSUM (8 banks): ph0/ph1/ph2 [128,1024] f32 = 2 banks each (bufs=1),
plg [1,1024] = 2 banks.  A short run of warmup matmuls on a memset
tile runs during the initial DMA wait to flip the PE HAM clock gate
(1.2 -> 2.4 GHz) before the real stream starts.

Logits strip is laid out [segment, node]: per 2-tile group the Lf
matmuls accumulate 1024 logits in plg (partition 0), one DVE copy
evacuates to SBUF and one DMA scatters them to 8 strip rows (one per
segment).  Per-segment softmax/argmax then runs on 4 chunks of
[128 segments, 128 nodes] with per-partition scalars:
  sc = strip+g; m = rowmax(sc); e,S = exp(strip) with accumulate;
  scr = (sc>=m)*iota -> am = rowmax (max-index tie-break, same as
  reference); ewin = sum((iota==am)*e); p = ewin/S.
Chunks 0-2 run during the matmul stream; only chunk 3 is tail work.
actions = am (local index, segments are exactly 128 nodes); host
derives shifted_actions = 128*seg + action.
"""

import sys

if "/opt/trn_rl_repo" not in sys.path:
    sys.path.insert(0, "/opt/trn_rl_repo")

import numpy as np

import concourse.bacc as bacc
import concourse.mybir as mybir
from concourse import tile
from concourse.bass_utils import run_bass_kernel_spmd

F32 = mybir.dt.float32
F16 = mybir.dt.float16
I32 = mybir.dt.int32
AF = mybir.ActivationFunctionType
OP = mybir.AluOpType
AX = mybir.AxisListType

N_CORES = 8
N = 524288
D = 128
H = 256
B_SEG = 4096
SEG = 128            # nodes per segment
N_LOC = N // N_CORES         # 65536 nodes per core
SEG_LOC = N_LOC // SEG       # 512 segments per core
CH = 128                     # segments per chunk (= partition dim)
NCHUNK = SEG_LOC // CH       # 4
NT_FULL = 128                # kept for test.py compat (tiles at tw=512)
TW = 512


def build(nt=NT_FULL, tw=TW, warmups=18):
    nc = bacc.Bacc("TRN2", target_bir_lowering=False, debug=False)
    ntile = N_LOC // tw          # tiles per core
    gtiles = 1024 // tw          # tiles per logits group (1024 nodes)
    ngrp = ntile // gtiles       # 64 groups
    grp_per_chunk = ngrp // NCHUNK   # 16
    pbufs = 1 if tw > 256 else 2

    xt_d = nc.dram_tensor("xt", [128, N_LOC], F16, kind="ExternalInput")
    w0_d = nc.dram_tensor("w0", [128, 256], F16, kind="ExternalInput")
    wc_d = nc.dram_tensor("wc", [128, 1026], F16, kind="ExternalInput")
    g_d = nc.dram_tensor("g", [128, 512], F32, kind="ExternalInput")
    out_d = nc.dram_tensor("out", [128, 8], F32, kind="ExternalOutput")

    with tile.TileContext(nc) as tc:
        with tc.tile_pool(name="const", bufs=1) as cpool, \
             tc.tile_pool(name="xp", bufs=5) as xpool, \
             tc.tile_pool(name="hp", bufs=2) as hpool, \
             tc.tile_pool(name="pp", bufs=1, space="PSUM") as ppool, \
             tc.tile_pool(name="sp", bufs=2) as spool:

            w0t = cpool.tile([128, 256], F16)
            wct = cpool.tile([128, 1026], F16)
            gt = cpool.tile([128, 512], F32)
            strip = cpool.tile([128, 512], F32)
            out_t = cpool.tile([128, 8], F32)
            warm16 = cpool.tile([128, 128], F16)
            nc.gpsimd.memset(warm16[:], 0)

            # Head DMAs: w0 on the Sync queue, x0 in parallel on the
            # Scalar (Activation) HW-DGE queue so the first matmul's two
            # dependencies transfer concurrently.
            nc.sync.dma_start(w0t[:], w0_d[:])
            xts = {}
            xts[0] = xpool.tile([128, tw], F16, tag="xt", name="xt0")
            nc.scalar.dma_start(xts[0][:], xt_d[:, 0:tw])
            xts[1] = xpool.tile([128, tw], F16, tag="xt", name="xt1")
            nc.sync.dma_start(xts[1][:], xt_d[:, tw:2 * tw])
            nc.sync.dma_start(wct[:], wc_d[:])
            w1 = wct[:, 0:512]
            w2 = wct[:, 512:1024]
            wf = wct[:, 1024:1026]

            iota_i = cpool.tile([128, 128], I32)
            nc.gpsimd.iota(iota_i[:], pattern=[[1, 128]], base=0,
                           channel_multiplier=0)
            iota128 = cpool.tile([128, 128], F32)
            nc.vector.tensor_copy(iota128[:], iota_i[:])

            # PE warmup: dependency-free matmuls fill the initial DMA
            # wait so the HAM clock gate is already 8/8 when the real
            # stream starts.  Reuses the ph2 PSUM allocation.
            wps = ppool.tile([128, 2 * tw], F32, tag="ph2", bufs=pbufs,
                             name="warm")
            for i in range(warmups):
                nc.tensor.matmul(wps[:, 0:128], warm16[:], warm16[:],
                                 start=True, stop=True)

            def emit_chunk(c):
                cs = CH * c
                stc = strip[:, cs:cs + CH]
                gc = gt[:, cs:cs + CH]
                sc = spool.tile([128, CH], F32, tag="sc", name=f"sc{c}")
                nc.vector.tensor_add(sc[:], stc, gc)
                e_c = spool.tile([128, CH], F32, tag="ec", name=f"ec{c}")
                s_c = spool.tile([128, 1], F32, tag="Sc", name=f"Sc{c}")
                nc.scalar.activation(e_c[:], stc, AF.Exp, accum_out=s_c[:])
                m_c = spool.tile([128, 1], F32, tag="mc", name=f"mc{c}")
                nc.vector.tensor_reduce(m_c[:], sc[:], axis=AX.X, op=OP.max)
                scr = spool.tile([128, CH], F32, tag="scr", name=f"scr{c}")
                nc.vector.scalar_tensor_tensor(scr[:], sc[:], m_c[:],
                                               iota128[:], op0=OP.is_ge,
                                               op1=OP.mult)
                nc.vector.tensor_reduce(out_t[:, 4 + c:5 + c], scr[:],
                                        axis=AX.X, op=OP.max)
                ew = spool.tile([128, CH], F32, tag="ew", name=f"ew{c}")
                ewin = spool.tile([128, 1], F32, tag="ewin", name=f"ewin{c}")
                nc.vector.scalar_tensor_tensor(ew[:], iota128[:],
                                               out_t[:, 4 + c:5 + c],
                                               e_c[:], op0=OP.is_equal,
                                               op1=OP.mult,
                                               accum_out=ewin[:])
                rcp = spool.tile([128, 1], F32, tag="rcp", name=f"rcp{c}")
                nc.vector.reciprocal(rcp[:], s_c[:])
                nc.vector.tensor_mul(out_t[:, c:c + 1], ewin[:], rcp[:])

            # 3-stage software pipeline: per iteration emit L0(t),
            # L1(t-1), L2(t-2), Lf(t-3).  Every relu evacuation then has
            # a full tile-window of queued matmuls between its producer
            # and its consumers/overwriters, so the single-buffered PSUM
            # tags never stall the Tensor queue.
            h0s, h1s, h2s = {}, {}, {}
            plg = None
            for tt in range(ntile + 3):
                if tt < ntile:
                    t = tt
                    if t in xts:
                        xt = xts.pop(t)
                    else:
                        xt = xpool.tile([128, tw], F16, tag="xt",
                                        name=f"xt{t}")
                        nc.sync.dma_start(xt[:], xt_d[:, t * tw:(t + 1) * tw])
                    if t == 4:
                        nc.sync.dma_start(gt[:], g_d[:])
                    ph0 = ppool.tile([128, 2 * tw], F32, tag="ph0",
                                     bufs=pbufs, name=f"ph0_{t}")
                    nc.tensor.matmul(ph0[:, 0:tw], w0t[:, 0:128], xt[:],
                                     start=True, stop=True)
                    nc.tensor.matmul(ph0[:, tw:2 * tw], w0t[:, 128:256],
                                     xt[:], start=True, stop=True)
                    h0 = hpool.tile([128, 2 * tw], F16, tag="h0", bufs=2,
                                    name=f"h0_{t}")
                    nc.scalar.activation(h0[:], ph0[:], AF.Relu)
                    h0s[t] = h0
                if 1 <= tt <= ntile:
                    t = tt - 1
                    h0 = h0s.pop(t)
                    h0a, h0b = h0[:, 0:tw], h0[:, tw:2 * tw]
                    ph1 = ppool.tile([128, 2 * tw], F32, tag="ph1",
                                     bufs=pbufs, name=f"ph1_{t}")
                    nc.tensor.matmul(ph1[:, 0:tw], w1[:, 0:128], h0a,
                                     start=True, stop=False)
                    nc.tensor.matmul(ph1[:, 0:tw], w1[:, 256:384], h0b,
                                     start=False, stop=True)
                    nc.tensor.matmul(ph1[:, tw:2 * tw], w1[:, 128:256], h0a,
                                     start=True, stop=False)
                    nc.tensor.matmul(ph1[:, tw:2 * tw], w1[:, 384:512], h0b,
                                     start=False, stop=True)
                    h1 = hpool.tile([128, 2 * tw], F16, tag="h1", bufs=2,
                                    name=f"h1_{t}")
                    nc.vector.tensor_scalar(h1[:], ph1[:], 0.0, None, OP.max)
                    h1s[t] = h1
                if 2 <= tt <= ntile + 1:
                    t = tt - 2
                    h1 = h1s.pop(t)
                    h1a, h1b = h1[:, 0:tw], h1[:, tw:2 * tw]
                    ph2 = ppool.tile([128, 2 * tw], F32, tag="ph2",
                                     bufs=pbufs, name=f"ph2_{t}")
                    nc.tensor.matmul(ph2[:, 0:tw], w2[:, 0:128], h1a,
                                     start=True, stop=False)
                    nc.tensor.matmul(ph2[:, 0:tw], w2[:, 256:384], h1b,
                                     start=False, stop=True)
                    nc.tensor.matmul(ph2[:, tw:2 * tw], w2[:, 128:256], h1a,
                                     start=True, stop=False)
                    nc.tensor.matmul(ph2[:, tw:2 * tw], w2[:, 384:512], h1b,
                                     start=False, stop=True)
                    h2 = hpool.tile([128, 2 * tw], F16, tag="h2", bufs=2,
                                    name=f"h2_{t}")
                    nc.scalar.activation(h2[:], ph2[:], AF.Relu)
                    h2s[t] = h2
                if tt < 3:
                    continue
                t = tt - 3
                h2 = h2s.pop(t)
                h2a, h2b = h2[:, 0:tw], h2[:, tw:2 * tw]
                if t % gtiles == 0:
                    plg = ppool.tile([1, 1024], F32, tag="plg", bufs=1,
                                     name=f"plg{t}")
                c0 = (t % gtiles) * tw
                nc.tensor.matmul(plg[0:1, c0:c0 + tw], wf[:, 0:1], h2a,
                                 start=True, stop=False)
                nc.tensor.matmul(plg[0:1, c0:c0 + tw], wf[:, 1:2], h2b,
                                 start=False, stop=True)
                if t % gtiles == gtiles - 1:
                    gidx = t // gtiles
                    stage = spool.tile([1, 1024], F32, tag="stage", bufs=2,
                                       name=f"stg{gidx}")
                    nc.vector.tensor_copy(stage[:], plg[:])
                    c = gidx // grp_per_chunk
                    r = 8 * (gidx % grp_per_chunk)
                    nc.sync.dma_start(strip[r:r + 8, CH * c:CH * (c + 1)],
                                      stage[0:1, :])
                    if gidx % grp_per_chunk == grp_per_chunk - 1:
                        emit_chunk(c)

            nc.sync.dma_start(out_d[:], out_t[:])

    nc.compile()
    return nc


_NC_CACHE = {}


def _get_nc(nt=NT_FULL):
    if nt not in _NC_CACHE:
        _NC_CACHE[nt] = build(nt)
    return _NC_CACHE[nt]


def _gumbel_host():
    import jax

    with jax.default_device(jax.devices("cpu")[0]):
        skey = jax.random.key(42)
        u = jax.random.uniform(skey, (N,), np.float32, 1e-20, 1.0)
        g = -np.log(-np.log(np.asarray(u)))
    return g.astype(np.float32)


def prep_in_maps(X, W0, b0, W1, b1, W2, b2, Wf, bf, g=None):
    # the graph folds the (always-zero) biases away; fail loudly otherwise
    for b in (b0, b1, b2):
        assert not np.any(np.asarray(b)), "nonzero MLP biases unsupported"
    if g is None:
        g = _gumbel_host()
    W1 = np.asarray(W1, np.float32)
    W2 = np.asarray(W2, np.float32)
    Wf = np.asarray(Wf, np.float32)
    w0 = np.asarray(W0, np.float32).astype(np.float16)          # [128,256]
    w1 = np.concatenate([W1[:128], W1[128:]], 1).astype(np.float16)
    w2 = np.concatenate([W2[:128], W2[128:]], 1).astype(np.float16)
    wf = np.stack([Wf[:128, 0], Wf[128:, 0]], 1).astype(np.float16)
    wc = np.ascontiguousarray(np.concatenate([w1, w2, wf], 1))  # [128,1026]
    X16 = np.asarray(X, np.float32).astype(np.float16)
    in_maps = []
    for c in range(N_CORES):
        xc = X16[c * N_LOC:(c + 1) * N_LOC]
        xtc = np.ascontiguousarray(xc.T)                        # [128,65536]
        gc = np.ascontiguousarray(
            g[c * N_LOC:(c + 1) * N_LOC]
            .reshape(NCHUNK, CH, SEG).transpose(1, 0, 2).reshape(128, 512))
        in_maps.append({
            "xt": xtc, "w0": np.ascontiguousarray(w0), "wc": wc, "g": gc,
        })
    return in_maps


def assemble(results):
    p = np.empty(B_SEG, np.float32)
    actions = np.empty(B_SEG, np.int32)
    for c in range(N_CORES):
        o = results[c]["out"]  # [128, 8]
        for cc in range(NCHUNK):
            lo = c * SEG_LOC + cc * CH
            p[lo:lo + CH] = o[:, cc]
            actions[lo:lo + CH] = np.rint(o[:, 4 + cc]).astype(np.int32)
    shifted = (np.arange(B_SEG, dtype=np.int32) * SEG + actions).astype(
        np.int32)
    return p, actions, shifted


def kernel(X, W0, b0, W1, b1, W2, b2, Wf, bf, batch, **kwargs):
    nc = _get_nc()
    in_maps = prep_in_maps(X, W0, b0, W1, b1, W2, b2, Wf, bf)
    res = run_bass_kernel_spmd(nc, in_maps, core_ids=list(range(N_CORES)))
    return assemble(res.results)
